# revision 18
# baseline (speedup 1.0000x reference)
"""Trainium2 Bass kernel for out = x @ expm(skew(angles)) + bias.

Strategy:
  - Data-parallel over the batch: x [16384, 512] is split into 8 shards of
    [2048, 512], one per NeuronCore. angles/bias are replicated.
  - Host only does layout: builds A = skew(angles) (fp32 exact + bf16),
    A+I, and the fp32r rounding of x^T; ships each core its x shard
    pre-transposed ([512, 2048]) so the contraction dim lands on SBUF
    partitions (pure marshaling, no FLOPs; the PE's matmul contracts over
    the partition dim, so x^T layout is required by the ISA). All linear
    algebra runs on-device.
  - Main matmul uses float32r operands (fp32 rounded to 11 mantissa
    bits): the PE streams fp32r at 1 column/cycle vs 2 half-rate
    LOW_HIGH passes for plain fp32 (4 cycle-equivalents). Host pre-rounds
    x with RNE so the PE's fp22 truncation is lossless; accumulation
    stays fp32 in PSUM.
  - The rotation is computed on-device via a degree-4 Taylor series in
    Paterson-Stockmeyer form (2 matmuls of 512^3, bf16 operands; the
    error they touch is only the O(A^2) terms, ~1e-5 of the output).
    Skew-symmetry supplies every transpose for free, and the signs are
    folded into the DVE coefficients so no negation pass is needed:
        N2 = A^T @ A = -A^2     (lhsT = A, rhs = A)
        B' = A + A^2/4          =  -N2/4  + A
        t3 = (I + A) + A^2/2    =  -N2/2  + (A+I)
        P2 = (-A^2) @ B' = -C'  (lhsT = N2 [symmetric], rhs = B')
        W  = t3 + C'/6          =  -P2/6  + t3
           = I + A + A^2/2 + A^3/6 + A^4/24
    Degree-4 truncation for ||A||_2 ~ 0.48 is ~1e-5 on W -> ~5e-5
    relative on the output, below the fp32r x rounding (~2.4e-4).
    Measured end-to-end relative error ~3e-4 (gate 2e-2).
  - The PE sits idle for ~3us at kernel start waiting for the A DMA; the
    HAM clock gate holds the first ~3.4us of matmul activity at 1.2 GHz.
    A short burst of no-op matmuls on a zeroed tile starts the HAM window
    during the DMA wait so more of the expm chain runs at 2.4 GHz.
  - Main loop: per 128-row tile of x, 4 accumulating fp32r matmuls of
    N=512 straight from the preloaded x^T slices; the final DVE op adds
    bias while moving PSUM -> SBUF.
"""

import numpy as np

import concourse.bacc as bacc
import concourse.bass as bass
import concourse.mybir as mybir
import concourse.tile as tile
from concourse.bass_utils import run_bass_kernel_spmd

DIM = 512
BATCH = 16384
N_CORES = 8
XB = BATCH // N_CORES          # rows per core
P = 128                        # partitions
KT = DIM // P                  # 4 k-tiles
MT = XB // P                   # 16 m-tiles per core
XC = 4                         # m-tiles per x DMA chunk
NWARM = 33                     # PE warmup matmuls during the A DMA wait
F32 = mybir.dt.float32
F32R = mybir.dt.float32r
BF16 = mybir.dt.bfloat16
F16 = mybir.dt.float16

_CACHE = {}


def build_bass():
    nc = bacc.Bacc("TRN2", target_bir_lowering=False, debug=False)

    # all operands arrive pre-tiled from the host ([P, KT, ...] /
    # chunk-major for x) so every load is one contiguous-per-partition DMA:
    # cheap single descriptor-gen op, no rearrange patterns
    xt_d = nc.dram_tensor("xt", [MT // XC, P, KT, P * XC], F32R, kind="ExternalInput")
    ai_d = nc.dram_tensor("ai", [P, KT, DIM], F32, kind="ExternalInput")
    arb_d = nc.dram_tensor("arb", [P, KT, DIM], BF16, kind="ExternalInput")
    db_d = nc.dram_tensor("db", [P, KT, DIM], BF16, kind="ExternalInput")
    biasr_d = nc.dram_tensor("biasr", [P, DIM], F32, kind="ExternalInput")
    out_d = nc.dram_tensor("out", [XB, DIM], F16, kind="ExternalOutput")

    AOP = mybir.AluOpType

    with tile.TileContext(nc) as tc:
        with (
            tc.tile_pool(name="const", bufs=1) as cpool,
            tc.tile_pool(name="xin", bufs=MT // XC) as xpool,
            tc.tile_pool(name="oout", bufs=6) as opool,
            tc.tile_pool(name="eps", bufs=4, space=bass.MemorySpace.PSUM) as eps,
            tc.tile_pool(name="ops", bufs=3, space=bass.MemorySpace.PSUM) as ops,
            tc.tile_pool(name="wps", bufs=1, space=bass.MemorySpace.PSUM) as wps,
        ):
            ai_sb = cpool.tile([P, KT, DIM], F32)    # A + I
            arb_sb = cpool.tile([P, KT, DIM], BF16)  # A (bf16)
            db_sb = cpool.tile([P, KT, DIM], BF16)   # I/2 + A/6
            biasr_sb = cpool.tile([P, DIM], F32)

            # The 16 SDMA engines FAIR-SHARE among in-flight transfers
            # (packet-granularity round-robin), and a SINGLE transfer only
            # sustains ~160 GB/s -- so neither full concurrency (everything
            # lands late together) nor full serialization (forfeits
            # bandwidth) works.  Stage the wire in deadline-ordered groups:
            #   t0: {arb, db, ai}  ->  on db: {x0, x1, bias}
            #   on ai: x2          ->  on x0: x3
            # "on Y" = a 1-element DVE link op that reads Y (so it fires at
            # Y's DMA completion) and writes the successor's corner (so the
            # successor's DMA waits for it).  Links sit early in the DVE
            # FIFO, before the psum-consuming ops.
            xch = [
                xpool.tile([P, KT, P * XC], F32R, tag="x", name=f"xc{c}")
                for c in range(MT // XC)
            ]
            nc.sync.dma_start(arb_sb[:, :, :], arb_d[:, :, :])
            nc.sync.dma_start(db_sb[:, :, :], db_d[:, :, :])
            # ai + x0 start when arb completes (link on ACT / DVE resp.)
            nc.scalar.copy(ai_sb[0:1, 0:1, 0:1], arb_sb[0:1, 0:1, 0:1])
            nc.sync.dma_start(ai_sb[:, :, :], ai_d[:, :, :])

            # ---- PE warmup: short junk matmuls on a zeroed tile while the
            # A DMA is in flight, starting the HAM activity window early so
            # the expm chain runs closer to 2.4 GHz.  N=128 keeps the tail
            # quantization small so the real work isn't queued behind them.
            warm_sb = cpool.tile([P, 2 * P], BF16)
            nc.vector.memset(warm_sb[:, :], 0.0)
            nc.vector.tensor_scalar_mul(
                xch[0][0:1, 0:1, 0:1], arb_sb[0:1, 0:1, 0:1], 0.0
            )
            nc.vector.tensor_scalar_mul(
                biasr_sb[0:1, 0:1], arb_sb[0:1, 0:1, 0:1], 0.0
            )
            nc.sync.dma_start(xch[0][:, :, :], xt_d[0, :, :, :])
            nc.sync.dma_start(biasr_sb[:, :], biasr_d[:, :])
            warm_ps = wps.tile([P, P], F32, tag="warm")
            for _ in range(NWARM):
                nc.tensor.matmul(
                    warm_ps[:, :],
                    warm_sb[:, :P],
                    warm_sb[:, P:],
                    start=True,
                    stop=True,
                )

            # ---- expm chain (replicated; bf16 operands) ----
            # Both 512^3 matmuls run i-major (outer loop over the 4 psum
            # groups, inner over the contraction tiles): each psum group
            # completes 3 groups before the phase ends, so the DVE/ACT ops
            # that consume it pipeline behind the PE instead of gating the
            # next phase, and the PE never idles (keeping the HAM clock
            # gate at 2.4 GHz).
            n2_sb = cpool.tile([P, KT, DIM], BF16)   # -A^2
            bp2_sb = cpool.tile([P, KT, DIM], BF16)  # B2 = I/2 + A/6 + A^2/24
            m_sb = cpool.tile([P, KT, DIM], F32R)    # W

            # The series is factored so only two DVE ops touch each psum
            # group:   W = (I + A) + A^2 @ B2,  B2 = I/2 + A/6 + A^2/24
            # (d = I/2 + A/6 comes from the host; signs fold into the
            # coefficients since the PE produces -A^2 = A^T A directly).
            # N2 = A^T @ A = -A^2; per-group handoffs right after each stop:
            #   bp2[i] (DVE) = d - N2/24 — the P2 moving operand
            #   n2[i]  (ACT) — the P2 stationary operand
            pss = []
            for i in range(KT):
                ps = eps.tile([P, DIM], F32, tag="eps")
                pss.append(ps)
            for i in range(KT):
                for t in range(KT):
                    nc.tensor.matmul(
                        pss[i][:, :],
                        arb_sb[:, t, P * i : P * (i + 1)],
                        arb_sb[:, t, :],
                        start=(t == 0),
                        stop=(t == KT - 1),
                    )
                nc.vector.scalar_tensor_tensor(
                    bp2_sb[:, i, :], pss[i][:, :], -1.0 / 24.0, db_sb[:, i, :],
                    AOP.mult, AOP.add,
                )
                if i == KT - 1:
                    # split so P2's first group (which only needs column
                    # block 0 of this tile as its stationary operand) isn't
                    # gated on the full-tile copy
                    nc.scalar.copy(n2_sb[:, i, :P], pss[i][:, :P])
                    nc.scalar.copy(n2_sb[:, i, P:], pss[i][:, P:])
                else:
                    nc.scalar.copy(n2_sb[:, i, :], pss[i][:, :])

            # x1/x2/x3 chained behind each other via ACT links (the ACT
            # queue is idle after its copies; each link fires exactly at the
            # predecessor DMA's completion)
            nc.scalar.copy(xch[1][0:1, 0:1, 0:1], xch[0][0:1, 0:1, 0:1])
            nc.sync.dma_start(xch[1][:, :, :], xt_d[1, :, :, :])
            nc.scalar.copy(xch[2][0:1, 0:1, 0:1], xch[1][0:1, 0:1, 0:1])
            nc.sync.dma_start(xch[2][:, :, :], xt_d[2, :, :, :])
            nc.scalar.copy(xch[3][0:1, 0:1, 0:1], xch[2][0:1, 0:1, 0:1])
            nc.sync.dma_start(xch[3][:, :, :], xt_d[3, :, :, :])

            # P2 = (-A^2) @ B2; W tile i (the main-loop rhs) emerges right
            # after psum group i stops:  W = (A+I) - P2
            pss2 = []
            for i in range(KT):
                ps = eps.tile([P, DIM], F32, tag="eps")
                pss2.append(ps)
            for i in range(KT):
                for t in range(KT):
                    nc.tensor.matmul(
                        pss2[i][:, :],
                        n2_sb[:, t, P * i : P * (i + 1)],
                        bp2_sb[:, t, :],
                        start=(t == 0),
                        stop=(t == KT - 1),
                    )
                nc.vector.scalar_tensor_tensor(
                    m_sb[:, i, :], pss2[i][:, :], -1.0, ai_sb[:, i, :],
                    AOP.mult, AOP.add,
                )


            # ---- main loop: out = x @ W + bias ----
            for mi in range(MT):
                xc = xch[mi // XC]
                mo = P * (mi % XC)
                ps = ops.tile([P, DIM], F32, tag="out")
                for kb in range(KT):
                    nc.tensor.matmul(
                        ps[:, :],
                        xc[:, kb, mo : mo + P],
                        m_sb[:, kb, :],
                        start=(kb == 0),
                        stop=(kb == KT - 1),
                    )
                ot = opool.tile([P, DIM], F16, tag="o")
                nc.vector.tensor_add(ot[:, :], ps[:, :], biasr_sb[:, :])
                nc.sync.dma_start(out_d[P * mi : P * (mi + 1), :], ot[:, :])

    nc.compile()
    return nc


def _get_nc():
    if "nc" not in _CACHE:
        _CACHE["nc"] = build_bass()
    return _CACHE["nc"]


def _round_fp32r(x):
    """Round-to-nearest-even to 11 mantissa bits (verified bit-exact
    against walrus's fp32_to_fp32r)."""
    b = np.ascontiguousarray(x, dtype=np.float32).view(np.uint32).astype(np.uint64)
    b = b + 0x7FF + ((b >> 12) & 1)
    return (b & np.uint64(0xFFFFF000)).astype(np.uint32).view(np.float32)


def _bf16(x):
    import ml_dtypes

    return np.asarray(x, dtype=np.float32).astype(ml_dtypes.bfloat16)


def _host_inputs(angles, bias):
    angles = np.asarray(angles, dtype=np.float32)
    bias = np.asarray(bias, dtype=np.float32)
    iu, ju = np.triu_indices(DIM, k=1)
    A = np.zeros((DIM, DIM), dtype=np.float32)
    A[iu, ju] = angles
    A[ju, iu] = -angles
    def tiled(mat):
        # [DIM, DIM] -> [P, KT, DIM] with tiled[p, t, :] = mat[128t + p, :]
        return np.ascontiguousarray(
            mat.reshape(KT, P, DIM).transpose(1, 0, 2)
        )

    return {
        "ai": tiled(A + np.eye(DIM, dtype=np.float32)),
        "arb": tiled(_bf16(A)),
        "db": tiled(_bf16(0.5 * np.eye(DIM, dtype=np.float32) + A / np.float32(6.0))),
        "biasr": np.ascontiguousarray(
            np.broadcast_to(bias.reshape(1, DIM), (P, DIM))
        ),
    }


def kernel(x, angles, bias, _profile=False):
    x = np.asarray(x, dtype=np.float32)
    # per-core x shards, pre-transposed and pre-tiled to chunk-major
    # [chunk, p, t, m] with element = x[512*chunk + m, 128*t + p], and
    # pre-rounded to fp32r so the PE's fp22 truncation is lossless
    # (layout only)
    xts = _round_fp32r(
        np.ascontiguousarray(
            x.reshape(N_CORES, MT // XC, P * XC, KT, P).transpose(0, 1, 4, 3, 2)
        )
    )
    shared = _host_inputs(angles, bias)
    nc = _get_nc()
    in_maps = [{"xt": xts[c], **shared} for c in range(N_CORES)]
    res = run_bass_kernel_spmd(
        nc, in_maps, list(range(N_CORES)), trace=bool(_profile)
    )
    _CACHE["last_result"] = res
    out = np.concatenate(
        [np.asarray(res.results[c]["out"]) for c in range(N_CORES)], axis=0
    )
    return out.astype(np.float32)
